# revision 4
# baseline (speedup 1.0000x reference)
"""Cross-attention kernel for Trainium2 (Bass/Tile), SPMD over 8 NeuronCores.

Reference computation (per batch b, all fp32):
    q = x1 @ Wq ; k = x2 @ Wk ; v = x2 @ Wv          # [S, D] each
    scores = q.T @ k                                  # [D, D], contracts S
    A = softmax(scores / 32, axis=-1)
    out = v @ A                                       # [S, D]

Algebraic refactor (minimal FLOPs, hardware-natural layouts):
    G      = x1.T @ x2            # [D, D], contracts S
    scores = Wq.T @ (G @ Wk)      # two 1024^3 GEMMs
    out    = x2 @ (Wv @ A)        # one 1024^3 GEMM + one big GEMM

Precision strategy: residual-split fp8 ("3-term") with DoubleRow matmuls.
Each operand X is represented as Xh + Xl (both fp8e4m3, Xl = fp8(X - Xh)),
and X@Y ~ Xh@Yh + Xh@Yl + Xl@Yh: three DoubleRow GEMMs at 0.5 cyc/row =
0.75x the PE time of one bf16 GEMM, with ~12 effective mantissa bits
(more accurate than bf16). Used for phases P1 (G), P2 (G@Wk),
P3 (Wq.T@T2), P7 (x2@WvA). P6 (Wv@A) runs in bf16: the attention
weights E are softmax outputs whose fp8 residual hits the e4m3 subnormal
floor, so splitting does not refine them.

All transposes (x2.T for P7, Wv.T for P6) and all fp8/bf16 casts +
DoubleRow K-pair packing of constant inputs are done on the HOST -- the
PE does zero transpose work and the DMAs move 1-byte/2-byte data.

Power-of-2 scale bookkeeping (exact):
    weights prescaled x32 (std ~1 in fp8);  PSUM G cast at 1/16;
    PSUM T2 (= 2 x true) cast at 1/32;  PSUM S = 2 x true scores so
    softmax uses exp scale 1/64;  E normalized by 1/r before bf16 cast;
    PSUM out = 32 x true, copied out at 1/32 into fp32.

Sharding: data-parallel over batch B=8 -> one batch element per core,
weights replicated. No collectives.
"""

import numpy as np
import ml_dtypes

import concourse.bass as bass
import concourse.mybir as mybir
import concourse.tile as tile
from concourse import bacc
from concourse.bass_utils import run_bass_kernel_spmd

B, S, D = 8, 4096, 1024
NS2 = S // 256   # 16 sequence k-pairs
ND = D // 128    # 8 feature tiles
NP = D // 256    # 4 feature k-pairs

F8 = ml_dtypes.float8_e4m3
BF16 = ml_dtypes.bfloat16
FP32 = mybir.dt.float32
F8D = mybir.dt.float8e4
BF16D = mybir.dt.bfloat16
DR = mybir.MatmulPerfMode.DoubleRow
AF = mybir.ActivationFunctionType
ALU = mybir.AluOpType

EXP_SCALE = 1.0 / 64.0   # PSUM S is 2x true scores; true scale is 1/32


def _build(reps=1, phases=7):
    nc = bacc.Bacc()

    x1ph = nc.dram_tensor("x1ph", [NS2, 128, 2, D], F8D, kind="ExternalInput")
    x1pl = nc.dram_tensor("x1pl", [NS2, 128, 2, D], F8D, kind="ExternalInput")
    x2ph = nc.dram_tensor("x2ph", [NS2, 128, 2, D], F8D, kind="ExternalInput")
    x2pl = nc.dram_tensor("x2pl", [NS2, 128, 2, D], F8D, kind="ExternalInput")
    x2tph = nc.dram_tensor("x2tph", [NP, 128, 2, S], F8D, kind="ExternalInput")
    x2tpl = nc.dram_tensor("x2tpl", [NP, 128, 2, S], F8D, kind="ExternalInput")
    wkph = nc.dram_tensor("wkph", [NP, 128, 2, D], F8D, kind="ExternalInput")
    wkpl = nc.dram_tensor("wkpl", [NP, 128, 2, D], F8D, kind="ExternalInput")
    wqph = nc.dram_tensor("wqph", [NP, 128, 2, D], F8D, kind="ExternalInput")
    wqpl = nc.dram_tensor("wqpl", [NP, 128, 2, D], F8D, kind="ExternalInput")
    wvt = nc.dram_tensor("wvt", [ND, 128, D], BF16D, kind="ExternalInput")
    out = nc.dram_tensor("out", [S, D], FP32, kind="ExternalOutput")

    with tile.TileContext(nc) as tc:
        with (
            tc.tile_pool(name="perm", bufs=56) as perm,
            tc.tile_pool(name="st", bufs=24) as stp,
            tc.tile_pool(name="ob", bufs=3) as obp,
            tc.tile_pool(name="ps", bufs=8, space="PSUM") as psp,
        ):
            for _rep in range(reps):
                # ---- persistent weight tiles ----
                wk_t = {}
                wq_t = {}
                for h, src in (("h", wkph), ("l", wkpl)):
                    for k in range(NP):
                        t = perm.tile([128, 2, D], F8D, tag="perm",
                                      name=f"wk{h}{k}")
                        nc.sync.dma_start(out=t[:], in_=src[k, :, :, :])
                        wk_t[h, k] = t
                for h, src in (("h", wqph), ("l", wqpl)):
                    for k in range(NP):
                        t = perm.tile([128, 2, D], F8D, tag="perm",
                                      name=f"wq{h}{k}")
                        nc.sync.dma_start(out=t[:], in_=src[k, :, :, :])
                        wq_t[h, k] = t
                wvt_t = []
                for dt in range(ND):
                    t = perm.tile([128, D], BF16D, tag="perm", name=f"wvt{dt}")
                    nc.sync.dma_start(out=t[:], in_=wvt[dt, :, :])
                    wvt_t.append(t)

                gdr = {(h, k): perm.tile([128, 2, D], F8D, tag="perm",
                                         name=f"g{h}{k}")
                       for h in "hl" for k in range(NP)}
                t2dr = {(h, k): perm.tile([128, 2, D], F8D, tag="perm",
                                          name=f"t2{h}{k}")
                        for h in "hl" for k in range(NP)}
                en_t = [perm.tile([128, D], BF16D, tag="perm", name=f"en{dt}")
                        for dt in range(ND)]
                wa = {(h, k): perm.tile([128, 2, D], F8D, tag="perm",
                                        name=f"wa{h}{k}")
                      for h in "hl" for k in range(NP)}

                # ---- Phase 1: G[p, q] = x2.T @ x1 (contract s), 3-term ----
                with (
                    tc.tile_pool(name="x2p", bufs=32) as x2pool,
                    tc.tile_pool(name="x1s", bufs=8) as x1pool,
                ):
                    x2_t = {}
                    for h, src in (("h", x2ph), ("l", x2pl)):
                        for sp in range(NS2):
                            t = x2pool.tile([128, 2, D], F8D, tag="x2p",
                                            name=f"x2{h}{sp}")
                            nc.sync.dma_start(out=t[:], in_=src[sp, :, :, :])
                            x2_t[h, sp] = t

                    for qh in range(2):
                        banks = [psp.tile([128, 512], FP32, tag="ps",
                                          name=f"g{qh}_{p}") for p in range(ND)]
                        for sp in range(NS2):
                            x1h = x1pool.tile([128, 2, 512], F8D, tag="x1s")
                            nc.sync.dma_start(
                                out=x1h[:],
                                in_=x1ph[sp, :, :, qh * 512:(qh + 1) * 512])
                            x1l = x1pool.tile([128, 2, 512], F8D, tag="x1s")
                            nc.sync.dma_start(
                                out=x1l[:],
                                in_=x1pl[sp, :, :, qh * 512:(qh + 1) * 512])
                            for pt in range(ND):
                                sl = slice(pt * 128, (pt + 1) * 128)
                                for ti, (lh, rh) in enumerate(
                                        ((x2_t["h", sp], x1h),
                                         (x2_t["h", sp], x1l),
                                         (x2_t["l", sp], x1h))):
                                    nc.tensor.matmul(
                                        banks[pt][:],
                                        lhsT=lh[:, :, sl], rhs=rh[:],
                                        start=(sp == 0 and ti == 0),
                                        stop=(sp == NS2 - 1 and ti == 2),
                                        perf_mode=DR)
                        qs = slice(qh * 512, (qh + 1) * 512)
                        for pt in range(ND):
                            hi = gdr["h", pt // 2][:, pt % 2, qs]
                            nc.scalar.activation(hi, banks[pt][:], AF.Copy,
                                                 scale=1.0 / 16.0)
                            nc.vector.scalar_tensor_tensor(
                                gdr["l", pt // 2][:, pt % 2, qs],
                                banks[pt][:], 1.0 / 16.0, hi,
                                op0=ALU.mult, op1=ALU.subtract)

                if phases < 2:
                    continue
                with tc.tile_pool(name="x2t", bufs=8) as x2tpool:
                    x2t_t = {}
                    for h, src in (("h", x2tph), ("l", x2tpl)):
                        for k in range(NP):
                            t = x2tpool.tile([128, 2, S], F8D, tag="x2t",
                                             name=f"x2t{h}{k}")
                            nc.sync.dma_start(out=t[:], in_=src[k, :, :, :])
                            x2t_t[h, k] = t

                    # ---- Phase 2: T2[q, e] = (G/16) @ 32Wk, 3-term ----
                    for qt in range(ND if phases >= 2 else 0):
                        qsl = slice(qt * 128, (qt + 1) * 128)
                        for eh in range(2):
                            esl = slice(eh * 512, (eh + 1) * 512)
                            ps = psp.tile([128, 512], FP32, tag="ps",
                                          name="t2ps")
                            for k in range(NP):
                                for ti, (lh, rh) in enumerate(
                                        (("h", "h"), ("h", "l"), ("l", "h"))):
                                    nc.tensor.matmul(
                                        ps[:],
                                        lhsT=gdr[lh, k][:, :, qsl],
                                        rhs=wk_t[rh, k][:, :, esl],
                                        start=(k == 0 and ti == 0),
                                        stop=(k == NP - 1 and ti == 2),
                                        perf_mode=DR)
                            hi = t2dr["h", qt // 2][:, qt % 2, esl]
                            nc.scalar.activation(hi, ps[:], AF.Copy,
                                                 scale=1.0 / 32.0)
                            nc.vector.scalar_tensor_tensor(
                                t2dr["l", qt // 2][:, qt % 2, esl],
                                ps[:], 1.0 / 32.0, hi,
                                op0=ALU.mult, op1=ALU.subtract)

                    # ---- Phase 3 + softmax: S = 32Wq.T @ T2' (= 2x true) ----
                    for dt in range(ND if phases >= 3 else 0):
                        dsl = slice(dt * 128, (dt + 1) * 128)
                        pss = []
                        for eh in range(2):
                            esl = slice(eh * 512, (eh + 1) * 512)
                            ps = psp.tile([128, 512], FP32, tag="ps",
                                          name=f"s{dt}_{eh}")
                            for k in range(NP):
                                for ti, (lh, rh) in enumerate(
                                        (("h", "h"), ("h", "l"), ("l", "h"))):
                                    nc.tensor.matmul(
                                        ps[:],
                                        lhsT=wq_t[lh, k][:, :, dsl],
                                        rhs=t2dr[rh, k][:, :, esl],
                                        start=(k == 0 and ti == 0),
                                        stop=(k == NP - 1 and ti == 2),
                                        perf_mode=DR)
                            pss.append(ps)
                        mxs = []
                        for eh in range(2):
                            mx = stp.tile([128, 1], FP32, tag="st")
                            nc.vector.reduce_max(mx[:], pss[eh][:],
                                                 axis=mybir.AxisListType.X)
                            mxs.append(mx)
                        mx = stp.tile([128, 1], FP32, tag="st")
                        nc.vector.tensor_max(mx[:], mxs[0][:], mxs[1][:])
                        nb = stp.tile([128, 1], FP32, tag="st")
                        nc.scalar.mul(nb[:], mx[:], -EXP_SCALE)
                        sms = []
                        for eh in range(2):
                            esl = slice(eh * 512, (eh + 1) * 512)
                            sm = stp.tile([128, 1], FP32, tag="st")
                            nc.scalar.activation(
                                en_t[dt][:, esl], pss[eh][:], AF.Exp,
                                bias=nb[:], scale=EXP_SCALE, accum_out=sm[:])
                            sms.append(sm)
                        sm = stp.tile([128, 1], FP32, tag="st")
                        nc.vector.tensor_add(sm[:], sms[0][:], sms[1][:])
                        rc = stp.tile([128, 1], FP32, tag="st")
                        nc.vector.reciprocal(rc[:], sm[:])
                        nc.vector.tensor_scalar_mul(en_t[dt][:], en_t[dt][:],
                                                    rc[:])

                    # ---- Phase 6: WvA[p, e] = 32Wv @ EN (bf16) ----
                    for eh in range(2 if phases >= 6 else 0):
                        esl = slice(eh * 512, (eh + 1) * 512)
                        banks = [psp.tile([128, 512], FP32, tag="ps",
                                          name=f"wa{eh}_{p}")
                                 for p in range(ND)]
                        for dt in range(ND):
                            for pt in range(ND):
                                nc.tensor.matmul(
                                    banks[pt][:],
                                    lhsT=wvt_t[dt][:, pt * 128:(pt + 1) * 128],
                                    rhs=en_t[dt][:, esl],
                                    start=(dt == 0), stop=(dt == ND - 1))
                        for pt in range(ND):
                            hi = wa["h", pt // 2][:, pt % 2, esl]
                            nc.scalar.activation(hi, banks[pt][:], AF.Copy,
                                                 scale=1.0)
                            nc.vector.scalar_tensor_tensor(
                                wa["l", pt // 2][:, pt % 2, esl],
                                banks[pt][:], 1.0, hi,
                                op0=ALU.mult, op1=ALU.subtract)

                    # ---- Phase 7: out = x2 @ WvA (= 32x true), 3-term ----
                    for si in range(S // 128 if phases >= 7 else 0):
                        ssl = slice(si * 128, (si + 1) * 128)
                        ob = obp.tile([128, D], FP32, tag="ob")
                        for eh in range(2):
                            esl = slice(eh * 512, (eh + 1) * 512)
                            ps = psp.tile([128, 512], FP32, tag="ps",
                                          name="ops")
                            for k in range(NP):
                                for ti, (lh, rh) in enumerate(
                                        (("h", "h"), ("h", "l"), ("l", "h"))):
                                    nc.tensor.matmul(
                                        ps[:],
                                        lhsT=x2t_t[lh, k][:, :, ssl],
                                        rhs=wa[rh, k][:, :, esl],
                                        start=(k == 0 and ti == 0),
                                        stop=(k == NP - 1 and ti == 2),
                                        perf_mode=DR)
                            nc.vector.tensor_scalar_mul(ob[:, esl], ps[:],
                                                        1.0 / 32.0)
                        nc.sync.dma_start(out=out[ssl, :], in_=ob[:])

    nc.finalize()
    return nc


def _split8(x):
    hi = x.astype(F8)
    lo = (x - hi.astype(np.float32)).astype(F8)
    return hi, lo


def _pack_pairs(x):
    """[R, C] (R = n*256) -> [n, 128, 2, C]: tile[kp, j, c] = x[(2k+j)*128+kp, c]"""
    n = x.shape[0] // 256
    return np.ascontiguousarray(
        x.reshape(n, 2, 128, x.shape[1]).transpose(0, 2, 1, 3))


def prepare_in_maps(x_1, x_2, W_query, W_key, W_value):
    x_1 = np.asarray(x_1, dtype=np.float32)
    x_2 = np.asarray(x_2, dtype=np.float32)
    wq32 = 32.0 * np.asarray(W_query, dtype=np.float32)
    wk32 = 32.0 * np.asarray(W_key, dtype=np.float32)
    wvt32 = np.ascontiguousarray((32.0 * np.asarray(W_value, np.float32)).T)

    wqh, wql = _split8(wq32)
    wkh, wkl = _split8(wk32)
    shared = {
        "wqph": _pack_pairs(wqh), "wqpl": _pack_pairs(wql),
        "wkph": _pack_pairs(wkh), "wkpl": _pack_pairs(wkl),
        "wvt": np.ascontiguousarray(
            wvt32.astype(BF16).reshape(ND, 128, D)),
    }
    in_maps = []
    for b in range(B):
        x1h, x1l = _split8(x_1[b])
        x2h, x2l = _split8(x_2[b])
        x2th = np.ascontiguousarray(x2h.T)
        x2tl = np.ascontiguousarray(x2l.T)
        in_maps.append({
            "x1ph": _pack_pairs(x1h), "x1pl": _pack_pairs(x1l),
            "x2ph": _pack_pairs(x2h), "x2pl": _pack_pairs(x2l),
            "x2tph": _pack_pairs(x2th), "x2tpl": _pack_pairs(x2tl),
            **shared,
        })
    return in_maps


_NC = None


def _get_nc():
    global _NC
    if _NC is None:
        _NC = _build()
    return _NC


def kernel(x_1, x_2, W_query, W_key, W_value, _results_hook=None):
    nc = _get_nc()
    in_maps = prepare_in_maps(x_1, x_2, W_query, W_key, W_value)
    res = run_bass_kernel_spmd(nc, in_maps, list(range(B)))
    if _results_hook is not None:
        _results_hook(res)
    return np.stack([res.results[b]["out"] for b in range(B)], axis=0)


# revision 9
# speedup vs baseline: 1.0808x; 1.0808x over previous
"""Cross-attention kernel for Trainium2 (Bass/Tile), SPMD over 8 NeuronCores.

Reference computation (per batch b, all fp32):
    q = x1 @ Wq ; k = x2 @ Wk ; v = x2 @ Wv          # [S, D] each
    scores = q.T @ k                                  # [D, D], contracts S
    A = softmax(scores / 32, axis=-1)
    out = v @ A                                       # [S, D]

Algebraic refactor (minimal FLOPs, hardware-natural layouts):
    G      = x1.T @ x2            # [D, D], contracts S
    scores = Wq.T @ (G @ Wk)      # two 1024^3 GEMMs
    out    = x2 @ (Wv @ A)        # one 1024^3 GEMM + one big GEMM

Precision strategy: residual-split fp8 ("3-term") with DoubleRow matmuls.
Each operand X is represented as Xh + Xl (both fp8e4m3, Xl = fp8(X - Xh)),
and X@Y ~ Xh@Yh + Xh@Yl + Xl@Yh: three DoubleRow GEMMs at 0.5 cyc/row =
0.75x the PE time of one bf16 GEMM, with ~12 effective mantissa bits
(more accurate than bf16). Used for phases P1 (G), P2 (G@Wk),
P3 (Wq.T@T2), P7 (x2@WvA). P6 (Wv@A) runs in bf16: the attention
weights E are softmax outputs whose fp8 residual hits the e4m3 subnormal
floor, so splitting does not refine them.

All transposes (x2.T for P7, Wv.T for P6) and all fp8/bf16 casts +
DoubleRow K-pair packing of constant inputs are done on the HOST -- the
PE does zero transpose work and the DMAs move 1-byte/2-byte data.

Schedule notes:
  - P1 runs as two p-half passes (PSUM holds G[p-half, :] = 8 banks of
    [128,512]); the x2 p-half streams per pass (8 MB each), x1 loads once
    during pass 0 and stays resident. This keeps the DMA need of every
    pass under its PE time instead of front-loading 16 MB.
  - Every stationary (lhsT) load is reused by 2-4 consecutive matmuls
    (both e-halves x both moving splits) to amortize LDWEIGHTS.

Power-of-2 scale bookkeeping (exact):
    weights prescaled x32 (std ~1 in fp8);  PSUM G cast at 1/16;
    PSUM T2 (= 2 x true) cast at 1/32;  PSUM S = 2 x true scores so
    softmax uses exp scale 1/64;  E normalized by 1/r before bf16 cast;
    PSUM out = 32 x true, copied out at 1/32 into fp32.

Sharding: data-parallel over batch B=8 -> one batch element per core,
weights replicated. No collectives.
"""

import numpy as np
import ml_dtypes

import concourse.bass as bass
import concourse.mybir as mybir
import concourse.tile as tile
from concourse import bacc
from concourse.bass_utils import run_bass_kernel_spmd

B, S, D = 8, 4096, 1024
NS2 = S // 256   # 16 sequence k-pairs
ND = D // 128    # 8 feature tiles
NP = D // 256    # 4 feature k-pairs

F8 = ml_dtypes.float8_e4m3
BF16 = ml_dtypes.bfloat16
FP32 = mybir.dt.float32
F8D = mybir.dt.float8e4
BF16D = mybir.dt.bfloat16
DR = mybir.MatmulPerfMode.DoubleRow
AF = mybir.ActivationFunctionType
ALU = mybir.AluOpType

EXP_SCALE = 1.0 / 64.0   # PSUM S is 2x true scores; true scale is 1/32

# (lhsT-half, rhs-half) term order: h-stationary serves hh+hl, l serves lh
TERMS = (("h", "h"), ("h", "l"), ("l", "h"))


def _build(reps=1, phases=7):
    nc = bacc.Bacc()

    x1ph = nc.dram_tensor("x1ph", [NS2, 128, 2, D], F8D, kind="ExternalInput")
    x1pl = nc.dram_tensor("x1pl", [NS2, 128, 2, D], F8D, kind="ExternalInput")
    # x2 pair-packed, pre-split into p-halves: [sp, kp, j, 512]
    x2p = {}
    for ph in range(2):
        for h in "hl":
            x2p[h, ph] = nc.dram_tensor(f"x2p{h}{ph}", [NS2, 128, 2, 512],
                                        F8D, kind="ExternalInput")
    x2tph = nc.dram_tensor("x2tph", [NP, 128, 2, S], F8D, kind="ExternalInput")
    x2tpl = nc.dram_tensor("x2tpl", [NP, 128, 2, S], F8D, kind="ExternalInput")
    wkph = nc.dram_tensor("wkph", [NP, 128, 2, D], F8D, kind="ExternalInput")
    wkpl = nc.dram_tensor("wkpl", [NP, 128, 2, D], F8D, kind="ExternalInput")
    wqph = nc.dram_tensor("wqph", [NP, 128, 2, D], F8D, kind="ExternalInput")
    wqpl = nc.dram_tensor("wqpl", [NP, 128, 2, D], F8D, kind="ExternalInput")
    wvt = nc.dram_tensor("wvt", [ND, 128, D], BF16D, kind="ExternalInput")
    # P7 computes out TRANSPOSED ([e, s]) so the small WvA operand can be
    # the stationary one (reused across all 32 s-chunks -> LDWEIGHTS
    # amortized 8-16x); the host un-transposes for free.
    out = nc.dram_tensor("out", [D, S], FP32, kind="ExternalOutput")

    with tile.TileContext(nc) as tc:
        with (
            tc.tile_pool(name="perm", bufs=56) as perm,
            tc.tile_pool(name="st", bufs=24) as stp,
            tc.tile_pool(name="ob", bufs=2) as obp,
            tc.tile_pool(name="ps", bufs=8, space="PSUM") as psp,
        ):
            for _rep in range(reps):
                gdr = {(h, k): perm.tile([128, 2, D], F8D, tag="perm",
                                         name=f"g{h}{k}")
                       for h in "hl" for k in range(NP)}
                t2dr = {(h, k): perm.tile([128, 2, D], F8D, tag="perm",
                                          name=f"t2{h}{k}")
                        for h in "hl" for k in range(NP)}
                en_t = [perm.tile([128, D], BF16D, tag="perm", name=f"en{dt}")
                        for dt in range(ND)]
                wa = {(h, k): perm.tile([128, 2, D], F8D, tag="perm",
                                        name=f"wa{h}{k}")
                      for h in "hl" for k in range(NP)}

                # ---- Phase 1: G[p, q] = x2.T @ x1 (contract s), 3-term ----
                # Two p-half passes; banks = (4 p-tiles) x (2 q-halves).
                wk_t = {}
                wq_t = {}
                with (
                    tc.tile_pool(name="x1r", bufs=32) as x1pool,
                    tc.tile_pool(name="x2s", bufs=8) as x2pool,
                ):
                    x1_t = {}
                    for ph in range(2):
                        banks = {}
                        for pt4 in range(4):
                            for qh in range(2):
                                banks[pt4, qh] = psp.tile(
                                    [128, 512], FP32, tag="ps",
                                    name=f"g{ph}_{pt4}_{qh}")
                        for sp in range(NS2):
                            if ph == 0:
                                for h, src in (("h", x1ph), ("l", x1pl)):
                                    t = x1pool.tile([128, 2, D], F8D,
                                                    tag="x1r",
                                                    name=f"x1{h}{sp}")
                                    nc.sync.dma_start(out=t[:],
                                                      in_=src[sp, :, :, :])
                                    x1_t[h, sp] = t
                            x2_t = {}
                            for h in "hl":
                                t = x2pool.tile([128, 2, 512], F8D, tag="x2s")
                                nc.sync.dma_start(
                                    out=t[:], in_=x2p[h, ph][sp, :, :, :])
                                x2_t[h] = t
                            for pt4 in range(4):
                                sl = slice(pt4 * 128, (pt4 + 1) * 128)
                                # lhsT 'h' serves hh+hl on both q-halves;
                                # lhsT 'l' serves lh on both q-halves.
                                for lh_half, rh_list in (
                                        ("h", ("h", "l")), ("l", ("h",))):
                                    for rh_half in rh_list:
                                        ti = TERMS.index((lh_half, rh_half))
                                        for qh in range(2):
                                            qsl = slice(qh * 512,
                                                        (qh + 1) * 512)
                                            nc.tensor.matmul(
                                                banks[pt4, qh][:],
                                                lhsT=x2_t[lh_half][:, :, sl],
                                                rhs=x1_t[rh_half, sp][
                                                    :, :, qsl],
                                                start=(sp == 0 and ti == 0),
                                                stop=(sp == NS2 - 1
                                                      and ti == 2),
                                                perf_mode=DR)
                        # wk/wq weight loads enter the DMA queue between
                        # the two x2 stream passes
                        if ph == 0:
                            for h, src in (("h", wkph), ("l", wkpl)):
                                for k in range(NP):
                                    t = perm.tile([128, 2, D], F8D,
                                                  tag="perm", name=f"wk{h}{k}")
                                    nc.sync.dma_start(out=t[:],
                                                      in_=src[k, :, :, :])
                                    wk_t[h, k] = t
                            for h, src in (("h", wqph), ("l", wqpl)):
                                for k in range(NP):
                                    t = perm.tile([128, 2, D], F8D,
                                                  tag="perm", name=f"wq{h}{k}")
                                    nc.sync.dma_start(out=t[:],
                                                      in_=src[k, :, :, :])
                                    wq_t[h, k] = t
                        for pt4 in range(4):
                            pt = ph * 4 + pt4
                            for qh in range(2):
                                qs = slice(qh * 512, (qh + 1) * 512)
                                hi = gdr["h", pt // 2][:, pt % 2, qs]
                                nc.scalar.activation(hi, banks[pt4, qh][:],
                                                     AF.Copy,
                                                     scale=1.0 / 16.0)
                                nc.vector.scalar_tensor_tensor(
                                    gdr["l", pt // 2][:, pt % 2, qs],
                                    banks[pt4, qh][:], 1.0 / 16.0, hi,
                                    op0=ALU.mult, op1=ALU.subtract)

                if phases < 2:
                    continue
                with tc.tile_pool(name="x2t", bufs=8) as x2tpool:
                    wvt_t = []
                    for dt in range(ND):
                        t = perm.tile([128, D], BF16D, tag="perm",
                                      name=f"wvt{dt}")
                        nc.sync.dma_start(out=t[:], in_=wvt[dt, :, :])
                        wvt_t.append(t)
                    x2t_t = {}
                    for h, src in (("h", x2tph), ("l", x2tpl)):
                        for k in range(NP):
                            t = x2tpool.tile([128, 2, S], F8D, tag="x2t",
                                             name=f"x2t{h}{k}")
                            nc.sync.dma_start(out=t[:], in_=src[k, :, :, :])
                            x2t_t[h, k] = t

                    # ---- Phase 2: T2[q, e] = (G/16) @ 32Wk, 3-term ----
                    for qt in range(ND if phases >= 2 else 0):
                        qsl = slice(qt * 128, (qt + 1) * 128)
                        pse = [psp.tile([128, 512], FP32, tag="ps",
                                        name=f"t2ps{eh}") for eh in range(2)]
                        for k in range(NP):
                            for lh_half, rh_list in (
                                    ("h", ("h", "l")), ("l", ("h",))):
                                for rh_half in rh_list:
                                    ti = TERMS.index((lh_half, rh_half))
                                    for eh in range(2):
                                        esl = slice(eh * 512, (eh + 1) * 512)
                                        nc.tensor.matmul(
                                            pse[eh][:],
                                            lhsT=gdr[lh_half, k][:, :, qsl],
                                            rhs=wk_t[rh_half, k][:, :, esl],
                                            start=(k == 0 and ti == 0),
                                            stop=(k == NP - 1 and ti == 2),
                                            perf_mode=DR)
                        for eh in range(2):
                            esl = slice(eh * 512, (eh + 1) * 512)
                            hi = t2dr["h", qt // 2][:, qt % 2, esl]
                            nc.scalar.activation(hi, pse[eh][:], AF.Copy,
                                                 scale=1.0 / 32.0)
                            nc.vector.scalar_tensor_tensor(
                                t2dr["l", qt // 2][:, qt % 2, esl],
                                pse[eh][:], 1.0 / 32.0, hi,
                                op0=ALU.mult, op1=ALU.subtract)

                    # ---- Phase 3 + softmax: S = 32Wq.T @ T2' (= 2x true) ----
                    for dt in range(ND if phases >= 3 else 0):
                        dsl = slice(dt * 128, (dt + 1) * 128)
                        pss = [psp.tile([128, 512], FP32, tag="ps",
                                        name=f"s{dt}_{eh}") for eh in range(2)]
                        for k in range(NP):
                            for lh_half, rh_list in (
                                    ("h", ("h", "l")), ("l", ("h",))):
                                for rh_half in rh_list:
                                    ti = TERMS.index((lh_half, rh_half))
                                    for eh in range(2):
                                        esl = slice(eh * 512, (eh + 1) * 512)
                                        nc.tensor.matmul(
                                            pss[eh][:],
                                            lhsT=wq_t[lh_half, k][:, :, dsl],
                                            rhs=t2dr[rh_half, k][:, :, esl],
                                            start=(k == 0 and ti == 0),
                                            stop=(k == NP - 1 and ti == 2),
                                            perf_mode=DR)
                        mxs = []
                        for eh in range(2):
                            mx = stp.tile([128, 1], FP32, tag="st")
                            nc.vector.reduce_max(mx[:], pss[eh][:],
                                                 axis=mybir.AxisListType.X)
                            mxs.append(mx)
                        mx = stp.tile([128, 1], FP32, tag="st")
                        nc.vector.tensor_max(mx[:], mxs[0][:], mxs[1][:])
                        nb = stp.tile([128, 1], FP32, tag="st")
                        nc.scalar.mul(nb[:], mx[:], -EXP_SCALE)
                        sms = []
                        for eh in range(2):
                            esl = slice(eh * 512, (eh + 1) * 512)
                            sm = stp.tile([128, 1], FP32, tag="st")
                            nc.scalar.activation(
                                en_t[dt][:, esl], pss[eh][:], AF.Exp,
                                bias=nb[:], scale=EXP_SCALE, accum_out=sm[:])
                            sms.append(sm)
                        sm = stp.tile([128, 1], FP32, tag="st")
                        nc.vector.tensor_add(sm[:], sms[0][:], sms[1][:])
                        rc = stp.tile([128, 1], FP32, tag="st")
                        nc.vector.reciprocal(rc[:], sm[:])
                        nc.vector.tensor_scalar_mul(en_t[dt][:], en_t[dt][:],
                                                    rc[:])

                    # ---- Phase 6: WvA[p, e] = 32Wv @ EN (bf16) ----
                    # Two p-half passes; banks = (4 p-tiles) x (2 e-halves).
                    for ph6 in range(2 if phases >= 6 else 0):
                        banks = {}
                        for pt4 in range(4):
                            for eh in range(2):
                                banks[pt4, eh] = psp.tile(
                                    [128, 512], FP32, tag="ps",
                                    name=f"wa{ph6}_{pt4}_{eh}")
                        for dt in range(ND):
                            for pt4 in range(4):
                                pt = ph6 * 4 + pt4
                                psl = slice(pt * 128, (pt + 1) * 128)
                                for eh in range(2):
                                    esl = slice(eh * 512, (eh + 1) * 512)
                                    nc.tensor.matmul(
                                        banks[pt4, eh][:],
                                        lhsT=wvt_t[dt][:, psl],
                                        rhs=en_t[dt][:, esl],
                                        start=(dt == 0), stop=(dt == ND - 1))
                        for pt4 in range(4):
                            pt = ph6 * 4 + pt4
                            for eh in range(2):
                                esl = slice(eh * 512, (eh + 1) * 512)
                                hi = wa["h", pt // 2][:, pt % 2, esl]
                                nc.scalar.activation(hi, banks[pt4, eh][:],
                                                     AF.Copy, scale=1.0)
                                nc.vector.scalar_tensor_tensor(
                                    wa["l", pt // 2][:, pt % 2, esl],
                                    banks[pt4, eh][:], 1.0, hi,
                                    op0=ALU.mult, op1=ALU.subtract)

                    # ---- Phase 7: outT[e, s] = (x2 @ WvA).T (= 32x true) ----
                    # lhsT = wa e-slices (stationary, reused over all 8
                    # s-chunks x both moving splits); rhs = x2T streaming.
                    for et in range(ND if phases >= 7 else 0):
                        esl = slice(et * 128, (et + 1) * 128)
                        banks = [psp.tile([128, 512], FP32, tag="ps",
                                          name=f"o{et}_{sc}")
                                 for sc in range(8)]
                        for k in range(NP):
                            for lh_half, rh_list in (
                                    ("h", ("h", "l")), ("l", ("h",))):
                                for rh_half in rh_list:
                                    ti = TERMS.index((lh_half, rh_half))
                                    for sc in range(8):
                                        ssl = slice(sc * 512, (sc + 1) * 512)
                                        nc.tensor.matmul(
                                            banks[sc][:],
                                            lhsT=wa[lh_half, k][:, :, esl],
                                            rhs=x2t_t[rh_half, k][:, :, ssl],
                                            start=(k == 0 and ti == 0),
                                            stop=(k == NP - 1 and ti == 2),
                                            perf_mode=DR)
                        for half in range(2):
                            ob = obp.tile([128, 2048], FP32, tag="ob")
                            for s4 in range(4):
                                sc = half * 4 + s4
                                nc.vector.tensor_scalar_mul(
                                    ob[:, s4 * 512:(s4 + 1) * 512],
                                    banks[sc][:], 1.0 / 32.0)
                            nc.sync.dma_start(
                                out=out[esl, half * 2048:(half + 1) * 2048],
                                in_=ob[:])

    nc.finalize()
    return nc


def _split8(x):
    hi = x.astype(F8)
    lo = (x - hi.astype(np.float32)).astype(F8)
    return hi, lo


def _pack_pairs(x):
    """[R, C] (R = n*256) -> [n, 128, 2, C]: tile[kp, j, c] = x[(2k+j)*128+kp, c]"""
    n = x.shape[0] // 256
    return np.ascontiguousarray(
        x.reshape(n, 2, 128, x.shape[1]).transpose(0, 2, 1, 3))


def prepare_in_maps(x_1, x_2, W_query, W_key, W_value):
    x_1 = np.asarray(x_1, dtype=np.float32)
    x_2 = np.asarray(x_2, dtype=np.float32)
    wq32 = 32.0 * np.asarray(W_query, dtype=np.float32)
    wk32 = 32.0 * np.asarray(W_key, dtype=np.float32)
    wvt32 = np.ascontiguousarray((32.0 * np.asarray(W_value, np.float32)).T)

    wqh, wql = _split8(wq32)
    wkh, wkl = _split8(wk32)
    shared = {
        "wqph": _pack_pairs(wqh), "wqpl": _pack_pairs(wql),
        "wkph": _pack_pairs(wkh), "wkpl": _pack_pairs(wkl),
        "wvt": np.ascontiguousarray(
            wvt32.astype(BF16).reshape(ND, 128, D)),
    }
    in_maps = []
    for b in range(B):
        x1h, x1l = _split8(x_1[b])
        x2h, x2l = _split8(x_2[b])
        x2th = np.ascontiguousarray(x2h.T)
        x2tl = np.ascontiguousarray(x2l.T)
        x2ph_p = _pack_pairs(x2h)
        x2pl_p = _pack_pairs(x2l)
        m = {
            "x1ph": _pack_pairs(x1h), "x1pl": _pack_pairs(x1l),
            "x2tph": _pack_pairs(x2th), "x2tpl": _pack_pairs(x2tl),
            **shared,
        }
        for ph in range(2):
            sl = slice(ph * 512, (ph + 1) * 512)
            m[f"x2ph{ph}"] = np.ascontiguousarray(x2ph_p[:, :, :, sl])
            m[f"x2pl{ph}"] = np.ascontiguousarray(x2pl_p[:, :, :, sl])
        in_maps.append(m)
    return in_maps


_NC = None


def _get_nc():
    global _NC
    if _NC is None:
        _NC = _build()
    return _NC


def kernel(x_1, x_2, W_query, W_key, W_value, _results_hook=None):
    nc = _get_nc()
    in_maps = prepare_in_maps(x_1, x_2, W_query, W_key, W_value)
    res = run_bass_kernel_spmd(nc, in_maps, list(range(B)))
    if _results_hook is not None:
        _results_hook(res)
    # device emits out transposed [e, s]; un-transpose on host
    return np.stack(
        [np.ascontiguousarray(res.results[b]["out"].T) for b in range(B)],
        axis=0)
